# revision 17
# baseline (speedup 1.0000x reference)
"""MSRSA multi-head attention kernel for 8 Trainium2 NeuronCores.

Strategy: data-parallel over batch (B=8 -> 1 batch element per core).
Per core, for its batch element b (everything transposed, scores are S_T[k,q]):
  Qt = (W_q/8) @ queries^T   [512,1024]
  Kt = W_k @ keys^T          [512,1024]
  V  = values @ W_v^T        [1024,512]  (rows masked by attention_mask)
  per (q-chunk, head): S_T = Kt_h^T Qt_h + la*A^T + ld*fp8(D^T-5)
    (bias injected by one fp8 DoubleRow matmul per k-tile: [A|D] planes)
  expS = exp(S_T) on ScalarE (ScalarE does ONLY exp in steady state)
  PV flipped: out[q,65] = sum_k expS[k,q-tile]^T ... stat=expS slice,
    mov=V_ext[k, 64+mask] -> column 64 is the softmax denominator, per-
    partition (q) aligned, so normalization is one reciprocal + per-
    partition tensor_scalar multiplies on VectorE (no PE broadcasts).
  attn[q,hd] transposed back via PE transposes (identity matmul) into
  attnT[hd,q], then out = attnT^T @ WoT per q-tile.

Perf model notes (simulator): matmul cost = out-columns x PE clock; the PE
clock ramps 1.2->2.4 GHz after ~3us of *continuous* execution and resets on
any stall, so the whole kernel is software-pipelined to keep the PE queue
dependency-free: PV lags scores by one head (exp chases in between),
projections are interleaved between early heads as filler, transposes/out-
projection of chunk 0 run inside chunk 1's head stream. fp8 DoubleRow gets
no discount but one DR matmul still beats two fp16 matmuls for the 2-plane
bias; its LDWEIGHTS is dear (~213ns) so idents loads are grouped 2/head.
"""

import contextlib

import numpy as np

import concourse.bass as bass
import concourse.mybir as mybir
import concourse.tile as tile
from concourse.bass_utils import run_bass_kernel_spmd

B, L, DIN, DM, H = 8, 1024, 256, 512, 8
DH = DM // H  # 64
P = 128
NKT = L // P          # 8 k-tiles
NQC = 2               # q chunks
QC = L // NQC         # 512
NQT = QC // P         # 4 q-tiles per chunk
F32 = mybir.dt.float32
F16 = mybir.dt.float16
F8 = mybir.dt.float8e4
DR = mybir.MatmulPerfMode.DoubleRow
EXP = mybir.ActivationFunctionType.Exp


def _emit(tc):
    nc = tc.nc

    def dram(name, shape, dtype=F16, kind="ExternalInput"):
        return nc.dram_tensor(name, shape, dtype, kind=kind).ap()

    qT = dram("qT", [DIN, L])
    kT = dram("kT", [DIN, L])
    vT = dram("vT", [DIN, L])
    wqT = dram("wqT", [DIN, DM])
    wkT = dram("wkT", [DIN, DM])
    wvT = dram("wvT", [DIN, DM])
    woT = dram("woT", [DM, DM])
    ad8 = dram("ad8", [NKT * P, 2 * L], F8)   # per row: [A | fp8(D-5)]
    idn8 = dram("idn8", [P, H * 2 * P], F8)   # per-head (la, ld) diag subtiles
    ident16 = dram("ident16", [P, P], F16)    # identity for PE transposes
    mask01 = dram("mask01", [P, NKT], F32)
    out = dram("out", [L, DM], F16, kind="ExternalOutput")

    ad8_r = ad8.rearrange("(t p) (j q) -> p t j q", p=P, q=L)

    with contextlib.ExitStack() as ctx:
        singles = ctx.enter_context(tc.tile_pool(name="singles", bufs=1))
        big = ctx.enter_context(tc.tile_pool(name="big", bufs=1))
        exps = ctx.enter_context(tc.tile_pool(name="exps", bufs=3))
        small = ctx.enter_context(tc.tile_pool(name="small", bufs=4))
        attnp = ctx.enter_context(tc.tile_pool(name="attnp", bufs=2))
        ostp = ctx.enter_context(tc.tile_pool(name="ostp", bufs=3))
        # PSUM: sp 2x2 banks (scores) + misc 2x1 bank (pv/transpose/outproj
        # /proj tiles, all <=1 bank) + aux 2x1 = 8 banks total
        spsum = ctx.enter_context(tc.tile_pool(name="spsum", bufs=2, space="PSUM"))
        misc = ctx.enter_context(tc.tile_pool(name="misc", bufs=2, space="PSUM"))
        aux = ctx.enter_context(tc.tile_pool(name="aux", bufs=2, space="PSUM"))

        # ---- SBUF-resident tensors ----
        ad_sb = big.tile([P, NKT, 2, L], F8, tag="ad")      # [A | Dhi] rows
        qt_sb = big.tile([P, 4, L], F16, tag="qt")          # Qt[mt*128+p, l]
        kt_sb = big.tile([P, 4, L], F16, tag="kt")
        vx_sb = big.tile([P, NKT, H, DH + 1], F16, tag="vx")  # V + mask col
        q_sb = big.tile([P, 2, L], F16, tag="qstage")       # staged inputs
        k_sb = big.tile([P, 2, L], F16, tag="kstage")
        v_sb = big.tile([P, 2, L], F16, tag="vstage")
        wq_sb = big.tile([P, 2, DM], F16, tag="wq")
        wk_sb = big.tile([P, 2, DM], F16, tag="wk")
        wv_sb = big.tile([P, 2, DM], F16, tag="wv")
        wo_sb = big.tile([P, 4, DM], F16, tag="wo")
        idents = singles.tile([P, H, 2, P], F8, tag="idents")
        id16_sb = singles.tile([P, P], F16, tag="id16")
        mask_sb = singles.tile([P, NKT], F32, tag="mask")

        warm = singles.tile([1, 2], F32, tag="warm")
        nc.vector.memset(warm[:], 0.0)
        nc.scalar.activation(out=warm[:], in_=warm[:], func=EXP)
        # PE warmup spin: dependency-free tiny matmuls keep the tensor
        # engine continuously busy from the end of the preamble so the
        # 1.2->2.4GHz ramp completes before the first projection fires.
        wmm = singles.tile([P, P], F16, tag="wmm")
        nc.vector.memset(wmm[:], 0.0)
        wps = misc.tile([P, QC], F32, tag="misc", name="warmps")
        for _ in range(46):
            nc.tensor.matmul(wps[:, 0:P], wmm[:], wmm[:],
                             start=True, stop=True)

        # ---- DMA issue order == dependency order, across 3 HWDGE queues ----
        # sync: q,k halves then v, wo.  scalar: weights, idents, mask.
        # vector: ad tiles (bias path).
        nc.sync.dma_start(out=q_sb[:],
                          in_=qT.rearrange("(t p) l -> p t l", p=P))
        nc.scalar.dma_start(out=wq_sb[:],
                            in_=wqT.rearrange("(t p) d -> p t d", p=P))
        nc.gpsimd.dma_start(
            out=idents[:], in_=idn8.rearrange("p (h j m) -> p h j m", h=H, j=2))
        # (idents rides gpsimd: its 994ns swdge overhead is hidden, only
        # consumer is head-0 bias which starts well after)
        nc.sync.dma_start(out=k_sb[:],
                          in_=kT.rearrange("(t p) l -> p t l", p=P))
        nc.scalar.dma_start(out=wk_sb[:],
                            in_=wkT.rearrange("(t p) d -> p t d", p=P))
        for t in range(NKT):
            eng = nc.sync if t % 2 == 0 else nc.scalar
            eng.dma_start(out=ad_sb[:, t, :, :], in_=ad8_r[:, t, :, :])
        nc.scalar.dma_start(out=wv_sb[:],
                            in_=wvT.rearrange("(t p) d -> p t d", p=P))
        nc.sync.dma_start(out=v_sb[:],
                          in_=vT.rearrange("(t p) l -> p t l", p=P))
        nc.gpsimd.dma_start(out=mask_sb[:], in_=mask01[:])
        nc.gpsimd.dma_start(out=id16_sb[:], in_=ident16[:])
        nc.gpsimd.dma_start(out=wo_sb[:],
                            in_=woT.rearrange("(t p) d -> p t d", p=P))

        # ---- projection emitters (interleaved into the head stream) ----
        def emit_proj(x_sb, w_sb, dst, mt):
            ps = spsum.tile([P, 2 * QC], F32, tag="sp", name=f"prj{mt}")
            for lc in range(NQC):
                for kt2 in range(2):
                    nc.tensor.matmul(
                        ps[:, lc * QC:(lc + 1) * QC],
                        w_sb[:, kt2, mt * P:(mt + 1) * P],
                        x_sb[:, kt2, lc * QC:(lc + 1) * QC],
                        start=(kt2 == 0), stop=(kt2 == 1),
                    )
            nc.vector.tensor_copy(out=dst[:, mt, :], in_=ps[:])

        def emit_qk_proj(mt):
            emit_proj(q_sb, wq_sb, qt_sb, mt)
            emit_proj(k_sb, wk_sb, kt_sb, mt)

        def emit_v_proj(lt):
            ps = misc.tile([P, DM], F32, tag="misc", name=f"vprj{lt}")
            for kt2 in range(2):
                nc.tensor.matmul(
                    ps[:],
                    v_sb[:, kt2, lt * P:(lt + 1) * P],
                    wv_sb[:, kt2, :],
                    start=(kt2 == 0), stop=(kt2 == 1),
                )
            nc.vector.tensor_scalar_mul(
                out=vx_sb[:, lt, :, 0:DH],
                in0=ps.rearrange("p (h d) -> p h d", h=H),
                scalar1=mask_sb[:, lt:lt + 1],
            )
            nc.vector.tensor_copy(
                out=vx_sb[:, lt, :, DH:DH + 1],
                in_=mask_sb[:, lt:lt + 1, None].to_broadcast((P, H, 1)),
            )

        # ---- attention emitters ----
        ex_tiles = {}
        pv_tiles = {}
        attn_tiles = {}

        def emit_scores(qc, h):
            """bias (DR fp8) + scores for head h, chunk qc; exp chases on
            ScalarE. Two groups of 2 k-tile-pairs each to amortize the
            idents LDWEIGHTS (2 loads/head)."""
            qs = slice(qc * QC, (qc + 1) * QC)
            hb = (h % 2) * DH
            ht = h // 2
            ex = exps.tile([P, NKT, QC], F16, tag="ex", name=f"ex{qc}_{h}")
            ex_tiles[(qc, h)] = ex
            for g in range(2):          # pair groups {0,1} and {2,3}
                sps = []
                for pp in range(2):     # bias for both pairs: 1 idents LDW
                    sp = spsum.tile([P, 2 * QC], F32, tag="sp",
                                    name=f"sp{qc}_{h}_{g}_{pp}")
                    sps.append(sp)
                    for i in range(2):
                        kt = (2 * g + pp) * 2 + i
                        nc.tensor.matmul(
                            sp[:, i * QC:(i + 1) * QC],
                            idents[:, h, :, :], ad_sb[:, kt, :, qs],
                            start=True, stop=False, perf_mode=DR,
                        )
                for pp in range(2):     # scores on top, then exp
                    sp = sps[pp]
                    for i in range(2):
                        kt = (2 * g + pp) * 2 + i
                        nc.tensor.matmul(
                            sp[:, i * QC:(i + 1) * QC],
                            kt_sb[hb:hb + DH, ht, kt * P:(kt + 1) * P],
                            qt_sb[hb:hb + DH, ht, qs],
                            start=False, stop=True,
                        )
                    nc.scalar.activation(
                        out=ex[:, (2 * g + pp) * 2:(2 * g + pp) * 2 + 2, :]
                        .rearrange("p a b -> p (a b)"),
                        in_=sp[:],
                        func=EXP,
                    )

        def emit_pv(qc, h):
            """flipped PV: stat=expS[k-tile, q-tile], mov=V_ext[k-tile, 65].
            pv[q, 0:64] = unnormalized attn, pv[q, 64] = denominator."""
            ex = ex_tiles.pop((qc, h))
            pv = misc.tile([P, NQT, P], F32, tag="misc",
                           name=f"pv{qc}_{h}")
            pv_tiles[(qc, h)] = pv
            for qt in range(NQT):
                for kt in range(NKT):
                    nc.tensor.matmul(
                        pv[:, qt, 0:DH + 1],
                        ex[:, kt, qt * P:(qt + 1) * P],
                        vx_sb[:, kt, h, :],
                        start=(kt == 0), stop=(kt == NKT - 1),
                    )

        def emit_norm(qc, h):
            """normalize on VectorE: one reciprocal + per-q-tile multiply."""
            pv = pv_tiles.pop((qc, h))
            if qc not in attn_tiles:
                attn_tiles[qc] = attnp.tile([P, NQT, DM], F16, tag="attn",
                                            name=f"attn{qc}")
            attn = attn_tiles[qc]
            rec = small.tile([P, NQT], F32, tag="rec", name=f"rec{qc}_{h}")
            nc.vector.reciprocal(out=rec[:], in_=pv[:, :, DH])
            for qt in range(NQT):
                nc.vector.tensor_scalar_mul(
                    out=attn[:, qt, h * DH:(h + 1) * DH],
                    in0=pv[:, qt, 0:DH],
                    scalar1=rec[:, qt:qt + 1],
                )

        attnT_tiles = {}

        def emit_transposes(qc, hdts):
            """transpose attn[q,hd] -> attnT[hd,q] (PE) for given hd-tiles."""
            attn = attn_tiles[qc]
            if qc not in attnT_tiles:
                attnT_tiles[qc] = attnp.tile([P, 4, QC], F16, tag="attnT",
                                             name=f"aT{qc}")
            attnT = attnT_tiles[qc]
            for hdt in hdts:
                tp = aux.tile([P, NQT, 2 * P], F16, tag="aux", name=f"tp{qc}{hdt}")
                for qt in range(NQT):
                    nc.tensor.transpose(
                        tp[:, qt, 0:P],
                        attn[:, qt, hdt * P:(hdt + 1) * P],
                        id16_sb[:],
                    )
                nc.vector.tensor_copy(
                    out=attnT[:, hdt, :].rearrange("p (a b) -> p a b", a=NQT),
                    in_=tp[:, :, 0:P],
                )

        def emit_outproj(qc):
            attn_tiles.pop(qc)
            attnT = attnT_tiles.pop(qc)
            for qt in range(NQT):
                ws = aux.tile([P, DM], F32, tag="aux", name=f"ws{qc}{qt}")
                for hdt in range(4):
                    nc.tensor.matmul(
                        ws[:],
                        attnT[:, hdt, qt * P:(qt + 1) * P],
                        wo_sb[:, hdt, :],
                        start=(hdt == 0), stop=(hdt == 3),
                    )
                ost = ostp.tile([P, DM], F16, tag="ost", name=f"ost{qc}{qt}")
                # alternate evac + DMA engines so the final drain pipelines
                if qt % 2 == 0:
                    nc.scalar.copy(out=ost[:], in_=ws[:])
                    nc.sync.dma_start(
                        out=out[qc * QC + qt * P:qc * QC + (qt + 1) * P, :],
                        in_=ost[:],
                    )
                else:
                    nc.vector.tensor_copy(out=ost[:], in_=ws[:])
                    nc.gpsimd.dma_start(
                        out=out[qc * QC + qt * P:qc * QC + (qt + 1) * P, :],
                        in_=ost[:],
                    )

        # ---- the pipelined stream ----
        # filler work (projections) keyed by item index; heads 0,1 need
        # qk-tile 0, heads 2,3 tile 1, etc.; V must be done before PV(h0).
        filler = {
            0: lambda: (emit_qk_proj(2),),
            1: lambda: [emit_v_proj(lt) for lt in range(NKT)],
            2: lambda: (emit_qk_proj(3),),
        }
        LAG = 2
        emit_proj(q_sb, wq_sb, qt_sb, 0)
        emit_proj(q_sb, wq_sb, qt_sb, 1)
        emit_proj(k_sb, wk_sb, kt_sb, 0)
        emit_proj(k_sb, wk_sb, kt_sb, 1)
        items = [(qc, h) for qc in range(NQC) for h in range(H)]
        for i, (qc, h) in enumerate(items):
            emit_scores(qc, h)
            if i in filler:
                filler[i]()
            if i >= LAG:
                pqc, ph = items[i - LAG]
                emit_pv(pqc, ph)
                emit_norm(pqc, ph)
            if (qc, h) == (1, 1):
                emit_transposes(0, range(4))
        # tail: drain remaining PV/norm; transposes for heads already normed
        # fill the wait on the last exps; only hdt3 waits on head 7.
        emit_transposes(1, range(2))
        emit_pv(1, H - 2)
        emit_norm(1, H - 2)
        emit_outproj(0)
        emit_pv(1, H - 1)
        emit_norm(1, H - 1)
        emit_transposes(1, [2, 3])
        emit_outproj(1)


def build_nc():
    from concourse import bacc

    nc = bacc.Bacc("TRN2", target_bir_lowering=False, debug=False)
    with tile.TileContext(nc) as tc:
        _emit(tc)
    nc.compile()
    return nc


_NC = None


def _get_nc():
    global _NC
    if _NC is None:
        _NC = build_nc()
    return _NC


def make_in_maps(queries, keys, values, attention_mask, adjacency_matrix,
                 distance_matrix, W_q, W_k, W_v, W_o, lambda_a, lambda_d):
    import ml_dtypes

    f = np.float32
    h16 = np.float16
    f8 = ml_dtypes.float8_e4m3
    c = np.ascontiguousarray
    wqT = c((W_q.astype(f) * f(0.125)).T).astype(h16)
    wkT = c(W_k.astype(f).T).astype(h16)
    wvT = c(W_v.astype(f).T).astype(h16)
    woT = c(W_o.astype(f).T).astype(h16)
    la8 = lambda_a.astype(f).astype(f8).astype(f)
    ld8 = lambda_d.astype(f).astype(f8).astype(f)
    idn = np.zeros((P, H, 2, P), dtype=f)
    rr = np.arange(P)
    for h in range(H):
        idn[rr, h, 0, rr] = la8[h]
        idn[rr, h, 1, rr] = ld8[h]
    idn8 = idn.reshape(P, H * 2 * P).astype(f8)
    ident16 = np.eye(P, dtype=h16)
    in_maps = []
    for b in range(B):
        # per k-tile block of 128 rows: [A | fp8(D-5)]; the -5 shift centers
        # D's fp8 range and cancels in softmax
        A8 = adjacency_matrix[b].astype(f).T.astype(f8)
        Dhi = (distance_matrix[b].astype(f).T - f(5.0)).astype(f8)
        ad = np.concatenate(
            [A8.reshape(NKT, P, L), Dhi.reshape(NKT, P, L)], axis=2
        )  # [NKT, P, 2L]
        in_maps.append({
            "qT": c(queries[b].astype(f).T).astype(h16),
            "kT": c(keys[b].astype(f).T).astype(h16),
            "vT": c(values[b].astype(f).T).astype(h16),
            "wqT": wqT, "wkT": wkT, "wvT": wvT, "woT": woT,
            "ad8": c(ad.reshape(NKT * P, 2 * L)),
            "mask01": c((attention_mask[b] > 0).astype(f).reshape(NKT, P).T),
            "idn8": idn8, "ident16": ident16,
        })
    return in_maps


def kernel(queries, keys, values, attention_mask, adjacency_matrix,
           distance_matrix, W_q, W_k, W_v, W_o, lambda_a, lambda_d, **kw):
    nc = _get_nc()
    in_maps = make_in_maps(queries, keys, values, attention_mask,
                           adjacency_matrix, distance_matrix,
                           W_q, W_k, W_v, W_o, lambda_a, lambda_d)
    res = run_bass_kernel_spmd(nc, in_maps, list(range(B)), **kw)
    outs = np.stack([res.results[i]["out"] for i in range(B)]).astype(np.float32)
    return outs


# revision 18
# speedup vs baseline: 1.0224x; 1.0224x over previous
"""MSRSA multi-head attention kernel for 8 Trainium2 NeuronCores.

Strategy: data-parallel over batch (B=8 -> 1 batch element per core).
Per core, for its batch element b (everything transposed, scores are S_T[k,q]):
  Qt = (W_q/8) @ queries^T   [512,1024]
  Kt = W_k @ keys^T          [512,1024]
  V  = values @ W_v^T        [1024,512]  (rows masked by attention_mask)
  per (q-chunk, head): S_T = Kt_h^T Qt_h + la*A^T + ld*fp8(D^T-5)
    (bias injected by one fp8 DoubleRow matmul per k-tile: [A|D] planes)
  expS = exp(S_T) on ScalarE (ScalarE does ONLY exp in steady state)
  PV flipped: out[q,65] = sum_k expS[k,q-tile]^T ... stat=expS slice,
    mov=V_ext[k, 64+mask] -> column 64 is the softmax denominator, per-
    partition (q) aligned, so normalization is one reciprocal + per-
    partition tensor_scalar multiplies on VectorE (no PE broadcasts).
  attn[q,hd] transposed back via PE transposes (identity matmul) into
  attnT[hd,q], then out = attnT^T @ WoT per q-tile.

Perf model notes (simulator): matmul cost = out-columns x PE clock; the PE
clock ramps 1.2->2.4 GHz after ~3us of *continuous* execution and resets on
any stall, so the whole kernel is software-pipelined to keep the PE queue
dependency-free: PV lags scores by one head (exp chases in between),
projections are interleaved between early heads as filler, transposes/out-
projection of chunk 0 run inside chunk 1's head stream. fp8 DoubleRow gets
no discount but one DR matmul still beats two fp16 matmuls for the 2-plane
bias; its LDWEIGHTS is dear (~213ns) so idents loads are grouped 2/head.
"""

import contextlib

import numpy as np

import concourse.bass as bass
import concourse.mybir as mybir
import concourse.tile as tile
from concourse.bass_utils import run_bass_kernel_spmd

B, L, DIN, DM, H = 8, 1024, 256, 512, 8
DH = DM // H  # 64
P = 128
NKT = L // P          # 8 k-tiles
NQC = 2               # q chunks
QC = L // NQC         # 512
NQT = QC // P         # 4 q-tiles per chunk
F32 = mybir.dt.float32
F16 = mybir.dt.float16
F8 = mybir.dt.float8e4
DR = mybir.MatmulPerfMode.DoubleRow
EXP = mybir.ActivationFunctionType.Exp


def _emit(tc):
    nc = tc.nc

    def dram(name, shape, dtype=F16, kind="ExternalInput"):
        return nc.dram_tensor(name, shape, dtype, kind=kind).ap()

    qT = dram("qT", [DIN, L])
    kT = dram("kT", [DIN, L])
    vT = dram("vT", [DIN, L])
    wqT = dram("wqT", [DIN, DM])
    wkT = dram("wkT", [DIN, DM])
    wvT = dram("wvT", [DIN, DM])
    woT = dram("woT", [DM, DM])
    ad8 = dram("ad8", [NKT * P, 2 * L], F8)   # per row: [A | fp8(D-5)]
    idn8 = dram("idn8", [P, H * 2 * P], F8)   # per-head (la, ld) diag subtiles
    ident16 = dram("ident16", [P, P], F16)    # identity for PE transposes
    mask01 = dram("mask01", [P, NKT], F32)
    out = dram("out", [L, DM], F16, kind="ExternalOutput")

    ad8_r = ad8.rearrange("(t p) (j q) -> p t j q", p=P, q=L)

    with contextlib.ExitStack() as ctx:
        singles = ctx.enter_context(tc.tile_pool(name="singles", bufs=1))
        big = ctx.enter_context(tc.tile_pool(name="big", bufs=1))
        exps = ctx.enter_context(tc.tile_pool(name="exps", bufs=3))
        small = ctx.enter_context(tc.tile_pool(name="small", bufs=4))
        attnp = ctx.enter_context(tc.tile_pool(name="attnp", bufs=2))
        ostp = ctx.enter_context(tc.tile_pool(name="ostp", bufs=3))
        # PSUM: sp 2x2 banks (scores) + misc 2x1 bank (pv/transpose/outproj
        # /proj tiles, all <=1 bank) + aux 2x1 = 8 banks total
        spsum = ctx.enter_context(tc.tile_pool(name="spsum", bufs=2, space="PSUM"))
        misc = ctx.enter_context(tc.tile_pool(name="misc", bufs=2, space="PSUM"))
        aux = ctx.enter_context(tc.tile_pool(name="aux", bufs=2, space="PSUM"))

        # ---- SBUF-resident tensors ----
        ad_sb = big.tile([P, NKT, 2, L], F8, tag="ad")      # [A | Dhi] rows
        qt_sb = big.tile([P, 4, L], F16, tag="qt")          # Qt[mt*128+p, l]
        kt_sb = big.tile([P, 4, L], F16, tag="kt")
        vx_sb = big.tile([P, NKT, H, DH + 1], F16, tag="vx")  # V + mask col
        q_sb = big.tile([P, 2, L], F16, tag="qstage")       # staged inputs
        k_sb = big.tile([P, 2, L], F16, tag="kstage")
        v_sb = big.tile([P, 2, L], F16, tag="vstage")
        wq_sb = big.tile([P, 2, DM], F16, tag="wq")
        wk_sb = big.tile([P, 2, DM], F16, tag="wk")
        wv_sb = big.tile([P, 2, DM], F16, tag="wv")
        wo_sb = big.tile([P, 4, DM], F16, tag="wo")
        idents = singles.tile([P, H, 2, P], F8, tag="idents")
        id16_sb = singles.tile([P, P], F16, tag="id16")
        mask_sb = singles.tile([P, NKT], F32, tag="mask")

        warm = singles.tile([1, 2], F32, tag="warm")
        nc.vector.memset(warm[:], 0.0)
        nc.scalar.activation(out=warm[:], in_=warm[:], func=EXP)
        # PE warmup spin: dependency-free tiny matmuls keep the tensor
        # engine continuously busy from the end of the preamble so the
        # 1.2->2.4GHz ramp completes before the first projection fires.
        wmm = singles.tile([P, P], F16, tag="wmm")
        nc.vector.memset(wmm[:], 0.0)
        wps = misc.tile([P, QC], F32, tag="misc", name="warmps")
        for _ in range(46):
            nc.tensor.matmul(wps[:, 0:P], wmm[:], wmm[:],
                             start=True, stop=True)

        # ---- DMA issue order == dependency order, across 3 HWDGE queues ----
        # sync: q,k halves then v, wo.  scalar: weights, idents, mask.
        # vector: ad tiles (bias path).
        nc.sync.dma_start(out=q_sb[:],
                          in_=qT.rearrange("(t p) l -> p t l", p=P))
        nc.scalar.dma_start(out=wq_sb[:],
                            in_=wqT.rearrange("(t p) d -> p t d", p=P))
        nc.gpsimd.dma_start(
            out=idents[:], in_=idn8.rearrange("p (h j m) -> p h j m", h=H, j=2))
        # (idents rides gpsimd: its 994ns swdge overhead is hidden, only
        # consumer is head-0 bias which starts well after)
        nc.sync.dma_start(out=k_sb[:],
                          in_=kT.rearrange("(t p) l -> p t l", p=P))
        nc.scalar.dma_start(out=wk_sb[:],
                            in_=wkT.rearrange("(t p) d -> p t d", p=P))
        for t in range(NKT):
            eng = nc.sync if t % 2 == 0 else nc.scalar
            eng.dma_start(out=ad_sb[:, t, :, :], in_=ad8_r[:, t, :, :])
        nc.scalar.dma_start(out=wv_sb[:],
                            in_=wvT.rearrange("(t p) d -> p t d", p=P))
        nc.sync.dma_start(out=v_sb[:],
                          in_=vT.rearrange("(t p) l -> p t l", p=P))
        nc.gpsimd.dma_start(out=mask_sb[:], in_=mask01[:])
        nc.gpsimd.dma_start(out=id16_sb[:], in_=ident16[:])
        nc.gpsimd.dma_start(out=wo_sb[:],
                            in_=woT.rearrange("(t p) d -> p t d", p=P))

        # ---- projection emitters (interleaved into the head stream) ----
        def emit_proj(x_sb, w_sb, dst, mt):
            for lc in range(NQC):
                ps = misc.tile([P, QC], F32, tag="misc",
                               name=f"prj{mt}_{lc}")
                for kt2 in range(2):
                    nc.tensor.matmul(
                        ps[:],
                        w_sb[:, kt2, mt * P:(mt + 1) * P],
                        x_sb[:, kt2, lc * QC:(lc + 1) * QC],
                        start=(kt2 == 0), stop=(kt2 == 1),
                    )
                nc.vector.tensor_copy(
                    out=dst[:, mt, lc * QC:(lc + 1) * QC], in_=ps[:])

        def emit_qk_proj(mt):
            emit_proj(q_sb, wq_sb, qt_sb, mt)
            emit_proj(k_sb, wk_sb, kt_sb, mt)

        def emit_v_proj(lt):
            ps = misc.tile([P, DM], F32, tag="misc", name=f"vprj{lt}")
            for kt2 in range(2):
                nc.tensor.matmul(
                    ps[:],
                    v_sb[:, kt2, lt * P:(lt + 1) * P],
                    wv_sb[:, kt2, :],
                    start=(kt2 == 0), stop=(kt2 == 1),
                )
            nc.vector.tensor_scalar_mul(
                out=vx_sb[:, lt, :, 0:DH],
                in0=ps.rearrange("p (h d) -> p h d", h=H),
                scalar1=mask_sb[:, lt:lt + 1],
            )
            nc.vector.tensor_copy(
                out=vx_sb[:, lt, :, DH:DH + 1],
                in_=mask_sb[:, lt:lt + 1, None].to_broadcast((P, H, 1)),
            )

        # ---- attention emitters ----
        ex_tiles = {}
        pv_tiles = {}
        attn_tiles = {}

        def emit_scores(qc, h):
            """bias (DR fp8) + scores for head h, chunk qc; exp chases on
            ScalarE. Two groups of 2 k-tile-pairs each to amortize the
            idents LDWEIGHTS (2 loads/head)."""
            qs = slice(qc * QC, (qc + 1) * QC)
            hb = (h % 2) * DH
            ht = h // 2
            ex = exps.tile([P, NKT, QC], F16, tag="ex", name=f"ex{qc}_{h}")
            ex_tiles[(qc, h)] = ex
            for g in range(2):          # pair groups {0,1} and {2,3}
                sps = []
                for pp in range(2):     # bias for both pairs: 1 idents LDW
                    sp = spsum.tile([P, 2 * QC], F32, tag="sp",
                                    name=f"sp{qc}_{h}_{g}_{pp}")
                    sps.append(sp)
                    for i in range(2):
                        kt = (2 * g + pp) * 2 + i
                        nc.tensor.matmul(
                            sp[:, i * QC:(i + 1) * QC],
                            idents[:, h, :, :], ad_sb[:, kt, :, qs],
                            start=True, stop=False, perf_mode=DR,
                        )
                for pp in range(2):     # scores on top, then exp
                    sp = sps[pp]
                    for i in range(2):
                        kt = (2 * g + pp) * 2 + i
                        nc.tensor.matmul(
                            sp[:, i * QC:(i + 1) * QC],
                            kt_sb[hb:hb + DH, ht, kt * P:(kt + 1) * P],
                            qt_sb[hb:hb + DH, ht, qs],
                            start=False, stop=True,
                        )
                    nc.scalar.activation(
                        out=ex[:, (2 * g + pp) * 2:(2 * g + pp) * 2 + 2, :]
                        .rearrange("p a b -> p (a b)"),
                        in_=sp[:],
                        func=EXP,
                    )

        def emit_pv(qc, h):
            """flipped PV: stat=expS[k-tile, q-tile], mov=V_ext[k-tile, 65].
            pv[q, 0:64] = unnormalized attn, pv[q, 64] = denominator."""
            ex = ex_tiles.pop((qc, h))
            pv = misc.tile([P, NQT, P], F32, tag="misc",
                           name=f"pv{qc}_{h}")
            pv_tiles[(qc, h)] = pv
            for qt in range(NQT):
                for kt in range(NKT):
                    nc.tensor.matmul(
                        pv[:, qt, 0:DH + 1],
                        ex[:, kt, qt * P:(qt + 1) * P],
                        vx_sb[:, kt, h, :],
                        start=(kt == 0), stop=(kt == NKT - 1),
                    )

        def emit_norm(qc, h):
            """normalize on VectorE: one reciprocal + per-q-tile multiply."""
            pv = pv_tiles.pop((qc, h))
            if qc not in attn_tiles:
                attn_tiles[qc] = attnp.tile([P, NQT, DM], F16, tag="attn",
                                            name=f"attn{qc}")
            attn = attn_tiles[qc]
            rec = small.tile([P, NQT], F32, tag="rec", name=f"rec{qc}_{h}")
            nc.vector.reciprocal(out=rec[:], in_=pv[:, :, DH])
            for qt in range(NQT):
                nc.vector.tensor_scalar_mul(
                    out=attn[:, qt, h * DH:(h + 1) * DH],
                    in0=pv[:, qt, 0:DH],
                    scalar1=rec[:, qt:qt + 1],
                )

        attnT_tiles = {}

        def emit_transposes(qc, hdts):
            """transpose attn[q,hd] -> attnT[hd,q] (PE) for given hd-tiles."""
            attn = attn_tiles[qc]
            if qc not in attnT_tiles:
                attnT_tiles[qc] = attnp.tile([P, 4, QC], F16, tag="attnT",
                                             name=f"aT{qc}")
            attnT = attnT_tiles[qc]
            for hdt in hdts:
                tp = aux.tile([P, NQT, 2 * P], F16, tag="aux", name=f"tp{qc}{hdt}")
                for qt in range(NQT):
                    nc.tensor.transpose(
                        tp[:, qt, 0:P],
                        attn[:, qt, hdt * P:(hdt + 1) * P],
                        id16_sb[:],
                    )
                nc.vector.tensor_copy(
                    out=attnT[:, hdt, :].rearrange("p (a b) -> p a b", a=NQT),
                    in_=tp[:, :, 0:P],
                )

        def emit_outproj(qc):
            attn_tiles.pop(qc)
            attnT = attnT_tiles.pop(qc)
            for qt in range(NQT):
                ws = aux.tile([P, DM], F32, tag="aux", name=f"ws{qc}{qt}")
                for hdt in range(4):
                    nc.tensor.matmul(
                        ws[:],
                        attnT[:, hdt, qt * P:(qt + 1) * P],
                        wo_sb[:, hdt, :],
                        start=(hdt == 0), stop=(hdt == 3),
                    )
                ost = ostp.tile([P, DM], F16, tag="ost", name=f"ost{qc}{qt}")
                # alternate evac + DMA engines so the final drain pipelines
                if qt % 2 == 0:
                    nc.scalar.copy(out=ost[:], in_=ws[:])
                    nc.sync.dma_start(
                        out=out[qc * QC + qt * P:qc * QC + (qt + 1) * P, :],
                        in_=ost[:],
                    )
                else:
                    nc.vector.tensor_copy(out=ost[:], in_=ws[:])
                    nc.gpsimd.dma_start(
                        out=out[qc * QC + qt * P:qc * QC + (qt + 1) * P, :],
                        in_=ost[:],
                    )

        # ---- the pipelined stream ----
        # filler work (projections) keyed by item index; heads 0,1 need
        # qk-tile 0, heads 2,3 tile 1, etc.; V must be done before PV(h0).
        filler = {
            1: lambda: [emit_v_proj(lt) for lt in range(NKT)],
            2: lambda: (emit_qk_proj(2),),
            4: lambda: (emit_qk_proj(3),),
        }
        LAG = 2
        emit_proj(q_sb, wq_sb, qt_sb, 0)
        emit_proj(q_sb, wq_sb, qt_sb, 1)
        emit_proj(k_sb, wk_sb, kt_sb, 0)
        emit_proj(k_sb, wk_sb, kt_sb, 1)
        items = [(qc, h) for qc in range(NQC) for h in range(H)]
        for i, (qc, h) in enumerate(items):
            emit_scores(qc, h)
            if i in filler:
                filler[i]()
            if i >= LAG:
                pqc, ph = items[i - LAG]
                emit_pv(pqc, ph)
                emit_norm(pqc, ph)
            if (qc, h) == (1, 1):
                emit_transposes(0, range(4))
        # tail: drain remaining PV/norm; transposes for heads already normed
        # fill the wait on the last exps; only hdt3 waits on head 7.
        emit_transposes(1, range(2))
        emit_pv(1, H - 2)
        emit_norm(1, H - 2)
        emit_outproj(0)
        emit_pv(1, H - 1)
        emit_norm(1, H - 1)
        emit_transposes(1, [2, 3])
        emit_outproj(1)


def build_nc():
    from concourse import bacc

    nc = bacc.Bacc("TRN2", target_bir_lowering=False, debug=False)
    with tile.TileContext(nc) as tc:
        _emit(tc)
    nc.compile()
    return nc


_NC = None


def _get_nc():
    global _NC
    if _NC is None:
        _NC = build_nc()
    return _NC


def make_in_maps(queries, keys, values, attention_mask, adjacency_matrix,
                 distance_matrix, W_q, W_k, W_v, W_o, lambda_a, lambda_d):
    import ml_dtypes

    f = np.float32
    h16 = np.float16
    f8 = ml_dtypes.float8_e4m3
    c = np.ascontiguousarray
    wqT = c((W_q.astype(f) * f(0.125)).T).astype(h16)
    wkT = c(W_k.astype(f).T).astype(h16)
    wvT = c(W_v.astype(f).T).astype(h16)
    woT = c(W_o.astype(f).T).astype(h16)
    la8 = lambda_a.astype(f).astype(f8).astype(f)
    ld8 = lambda_d.astype(f).astype(f8).astype(f)
    idn = np.zeros((P, H, 2, P), dtype=f)
    rr = np.arange(P)
    for h in range(H):
        idn[rr, h, 0, rr] = la8[h]
        idn[rr, h, 1, rr] = ld8[h]
    idn8 = idn.reshape(P, H * 2 * P).astype(f8)
    ident16 = np.eye(P, dtype=h16)
    in_maps = []
    for b in range(B):
        # per k-tile block of 128 rows: [A | fp8(D-5)]; the -5 shift centers
        # D's fp8 range and cancels in softmax
        A8 = adjacency_matrix[b].astype(f).T.astype(f8)
        Dhi = (distance_matrix[b].astype(f).T - f(5.0)).astype(f8)
        ad = np.concatenate(
            [A8.reshape(NKT, P, L), Dhi.reshape(NKT, P, L)], axis=2
        )  # [NKT, P, 2L]
        in_maps.append({
            "qT": c(queries[b].astype(f).T).astype(h16),
            "kT": c(keys[b].astype(f).T).astype(h16),
            "vT": c(values[b].astype(f).T).astype(h16),
            "wqT": wqT, "wkT": wkT, "wvT": wvT, "woT": woT,
            "ad8": c(ad.reshape(NKT * P, 2 * L)),
            "mask01": c((attention_mask[b] > 0).astype(f).reshape(NKT, P).T),
            "idn8": idn8, "ident16": ident16,
        })
    return in_maps


def kernel(queries, keys, values, attention_mask, adjacency_matrix,
           distance_matrix, W_q, W_k, W_v, W_o, lambda_a, lambda_d, **kw):
    nc = _get_nc()
    in_maps = make_in_maps(queries, keys, values, attention_mask,
                           adjacency_matrix, distance_matrix,
                           W_q, W_k, W_v, W_o, lambda_a, lambda_d)
    res = run_bass_kernel_spmd(nc, in_maps, list(range(B)), **kw)
    outs = np.stack([res.results[i]["out"] for i in range(B)]).astype(np.float32)
    return outs
